# revision 3
# baseline (speedup 1.0000x reference)
"""Trainium2 Bass kernel for nn_ModelNew_3556232921828 (dense_cnn).

The reference computes:
    y = conv_transpose(x, w) + b            (finite for all finite inputs)
    s = exp(y - y)                          == 1 exactly (IEEE: y-y == +0)
    out = sigmoid(SCALE * s)                == sigmoid(2.0), a constant

So the output is the constant sigmoid(2.0) at every element, independent
of the (finite) input values.  The memory-optimal kernel therefore only
has to materialize the 16x64x128x128 f32 output in DRAM: each of the 8
cores fills a small [128, 512] SBUF tile with sigmoid(2.0) (DVE memset,
~0.5 us) and streams it to its 8 MiB output shard (batch dim sharded 2
per core) with one stride-0-source HWDGE DMA at the ~380 GB/s/core HBM
write roofline (~22 us).
"""

import numpy as np

import concourse.bass as bass
import concourse.mybir as mybir
from concourse.bass_utils import run_bass_kernel_spmd

N_CORES = 8
OUT_SHAPE = (16, 64, 128, 128)  # full output, f32
SHARD_B = OUT_SHAPE[0] // N_CORES  # 2 batches per core

# per-core shard = 2*64*128*128 f32 = 8 MiB = REP x [P, TILE_F] tiles
P = 128
TILE_F = 512
REP = (SHARD_B * OUT_SHAPE[1] * OUT_SHAPE[2] * OUT_SHAPE[3]) // (P * TILE_F)

SIGMOID_2 = float(1.0 / (1.0 + np.exp(np.float64(-2.0))))

_cached = {}


def _build() -> bass.Bass:
    nc = bass.Bass()
    out = nc.declare_dram_parameter(
        "out", [REP, P, TILE_F], mybir.dt.float32, isOutput=True
    )
    with (
        nc.Block(no_gpsimd_drain=True) as block,
        nc.semaphore("fill_sem") as fill_sem,
        nc.semaphore("dma_sem") as dma_sem,
        nc.sbuf_tensor("ctile", [P, TILE_F], mybir.dt.float32) as ctile,
    ):

        @block.vector
        def _(vector):
            vector.memset(ctile[:], SIGMOID_2).then_inc(fill_sem, 1)

        @block.sync
        def _(sync):
            sync.wait_ge(fill_sem, 1)
            src = ctile[:].unsqueeze(1).broadcast_to([P, REP, TILE_F])
            sync.dma_start(out=out[:], in_=src).then_inc(dma_sem, 16)
            sync.wait_ge(dma_sem, 16)

    return nc


def _run(trace: bool = False, **kwargs):
    if "nc" not in _cached:
        _cached["nc"] = _build()
    in_maps = [{} for _ in range(N_CORES)]
    try:
        return run_bass_kernel_spmd(
            _cached["nc"], in_maps, list(range(N_CORES)), trace=trace, **kwargs
        )
    except (ModuleNotFoundError, ImportError):
        # BASS_TRACE set but the axon NTFF profile hook isn't importable in
        # this environment — rerun without tracing rather than failing.
        import os

        os.environ["BASS_NEVER_TRACE"] = "1"
        return run_bass_kernel_spmd(
            _cached["nc"], in_maps, list(range(N_CORES)), trace=False, **kwargs
        )


def kernel(
    x: np.ndarray, weight: np.ndarray = None, bias: np.ndarray = None, **_
) -> np.ndarray:
    res = _run()
    shards = [
        r["out"].reshape(SHARD_B, OUT_SHAPE[1], OUT_SHAPE[2], OUT_SHAPE[3])
        for r in res.results
    ]
    return np.concatenate(shards, axis=0)
